# revision 18
# baseline (speedup 1.0000x reference)
"""Trainium2 Bass kernel for nn_CombinedModel (NMS detection + ROI classifier).

Strategy (v2, bf16):
  NMS/top-k/ROI selection host-side (tiny). Conv+pool+8-head GEMM on 8 cores.

  Sharding: pooled-row sharding. Core c computes conv rows y in [8c,8c+8)
  (pooled rows py in [4c,4c+4)) of all 304 (padded) ROIs == the k-slice
  {(oc,py,px): py in [4c,4c+4)} (2048 wide) of the 16384-deep W1 contraction.

  Conv as ONE 91-deep matmul per 512 cols: contraction rows = (y10, c3, kx3)
  plus an all-ones row that carries b_conv, with a zero-padded stationary
  [91, 128] whose output columns are (b2, a4, oc16) (y'=2a+b). The moving
  tensor t[(y,c,kx), r, x] holds kx-shifted image rows. 19 PSUM tiles
  [128, 16rois*32xh*2xpar].

  Pool: xpair-max drain from PSUM (DVE tensor_reduce or Act relu + max),
  then fused b-max+relu (scalar_tensor_tensor max0/max) into the two
  pooled2 partition halves (px bit 4).
  pooled2 [128=(px>=16, a, oc), 304 r, 16 q=px%16] bf16.

  GEMM: per (h,q): matmul(w1[128,128] stationary, pooled2[:,:,q] moving),
  16 q accumulated in PSUM; partials [128,8h,304] cast to bf16, exchanged
  with an AllToAll (head h partials -> core h), summed locally, then
  relu+b1, per-core head matmul [128,40], +b2, *keep.
"""
import os
import numpy as np
import ml_dtypes

BF16 = ml_dtypes.bfloat16

N_CORES = 8
R = 304            # 300 rois padded to 19*16
NRB = 19           # conv row blocks of 16 rois
IMG = 640
INP = 64
CONF = 0.25
IOU = 0.45
K = 300
PROV, ALPHA, AD = 38, 25, 35
OUTW = 40          # padded per-core head width

_CACHE = {}


def _build_bass():
    import concourse.bacc as bacc
    import concourse.mybir as mybir
    import concourse.tile as tile

    nc = bacc.Bacc("TRN2", target_bir_lowering=False, debug=False,
                   num_devices=N_CORES)
    f32 = mybir.dt.float32
    bf16 = mybir.dt.bfloat16
    AF = mybir.ActivationFunctionType
    MAX = mybir.AluOpType.max

    tmov = nc.dram_tensor("tmov", [91, R, 64], bf16, kind="ExternalInput").ap()
    w91 = nc.dram_tensor("w91", [91, 128], bf16, kind="ExternalInput").ap()
    w1s = nc.dram_tensor("w1s", [8, 128, 16, 128], bf16, kind="ExternalInput").ap()
    b1c = nc.dram_tensor("b1c", [128, 1], f32, kind="ExternalInput").ap()
    w2 = nc.dram_tensor("w2", [128, OUTW], bf16, kind="ExternalInput").ap()
    b2 = nc.dram_tensor("b2", [OUTW, 1], f32, kind="ExternalInput").ap()
    keepm = nc.dram_tensor("keepm", [OUTW, R], f32, kind="ExternalInput").ap()
    out = nc.dram_tensor("out", [OUTW, R], f32, kind="ExternalOutput").ap()

    with tile.TileContext(nc) as tc:
        with (
            tc.tile_pool(name="const", bufs=1) as cpool,
            tc.tile_pool(name="psum", bufs=1, space="PSUM") as psum,
            tc.tile_pool(name="work", bufs=2) as work,
            tc.tile_pool(name="dram", bufs=1, space="DRAM") as dpool,
        ):
            # conv stationary + first moving chunks go first: they gate rblk 0
            qeng = [nc.sync, nc.scalar]
            w91_sb = cpool.tile([91, 128], bf16)
            nc.sync.dma_start(w91_sb[:], w91[:])
            t_sb = cpool.tile([91, R * 64], bf16)
            rchunks = [(0, 16), (16, 16), (32, 32), (64, 48), (112, 48),
                       (160, 48), (208, 48), (256, 48)]
            for i, (r0, nr) in enumerate(rchunks[:2]):
                qeng[i % 2].dma_start(t_sb[:, r0 * 64:(r0 + nr) * 64],
                                      tmov[:, r0:r0 + nr, :])
            b1c_sb = cpool.tile([128, 1], f32)
            nc.scalar.dma_start(b1c_sb[:], b1c[:])
            w2_sb = cpool.tile([128, OUTW], bf16)
            nc.scalar.dma_start(w2_sb[:], w2[:])
            b2_sb = cpool.tile([OUTW, 1], f32)
            nc.scalar.dma_start(b2_sb[:], b2[:])
            keep_sb = cpool.tile([OUTW, R], f32)
            nc.scalar.dma_start(keep_sb[:], keepm[:])
            for i, (r0, nr) in enumerate(rchunks[2:]):
                qeng[i % 2].dma_start(t_sb[:, r0 * 64:(r0 + nr) * 64],
                                      tmov[:, r0:r0 + nr, :])
            # all 8 heads' W1 slices resident
            w1all = cpool.tile([128, 8, 16, 128], bf16)
            for h in range(8):
                qeng[(h + 1) % 2].dma_start(w1all[:, h, :, :], w1s[h])

            pooled2 = cpool.tile([128, R, 16], bf16)
            pooledQ = cpool.tile([128, 16, R], bf16)
            parts = cpool.tile([128, 8, R], bf16)

            # ---- conv + maxpool + relu ----
            # ps free layout per rblk: (r16, xpar2, xh32)
            for j in range(NRB):
                ps = psum.tile([128, 16, 2, 32], f32, tag="cv", bufs=2)
                psf = ps.rearrange("p a b c -> p (a b c)")
                nc.tensor.matmul(psf[:, 0:512], w91_sb[:],
                                 t_sb[:, j * 1024:j * 1024 + 512],
                                 start=True, stop=True)
                nc.tensor.matmul(psf[:, 512:1024], w91_sb[:],
                                 t_sb[:, j * 1024 + 512:(j + 1) * 1024],
                                 start=True, stop=True)
                # drain PSUM -> bf16 SBUF with relu (contiguous Act write)
                craw = work.tile([128, 16, 2, 32], bf16, tag="craw")
                nc.scalar.activation(
                    craw.rearrange("p a b c -> p (a b c)"), psf[:], AF.Relu)
                # xpair max per partition half (packed bf16, DVE 2x)
                xmA = work.tile([64, 16, 32], bf16, tag="xmA")
                xmB = work.tile([64, 16, 32], bf16, tag="xmB")
                nc.vector.tensor_tensor(out=xmA[:], in0=craw[0:64, :, 0, :],
                                        in1=craw[0:64, :, 1, :], op=MAX)
                nc.vector.tensor_tensor(out=xmB[:], in0=craw[64:128, :, 0, :],
                                        in1=craw[64:128, :, 1, :], op=MAX)
                # b-max into the two pooled2 partition halves (r-major, packed)
                nc.vector.tensor_tensor(
                    out=pooled2[0:64, 16 * j:16 * j + 16, :],
                    in0=xmA[:, :, 0:16], in1=xmB[:, :, 0:16], op=MAX)
                nc.vector.tensor_tensor(
                    out=pooled2[64:128, 16 * j:16 * j + 16, :],
                    in0=xmA[:, :, 16:32], in1=xmB[:, :, 16:32], op=MAX)
                # transpose this rblk to q-major for the GEMM (idle Pool engine)
                nc.gpsimd.tensor_copy(
                    pooledQ[:, :, 16 * j:16 * j + 16],
                    pooled2[:, 16 * j:16 * j + 16, :].rearrange("p r q -> p q r"))

            # ---- 8-head GEMM over this core's 2048-deep W1 slice ----
            cc_in = dpool.tile([8, 128, R], bf16)
            for h in range(8):
                ph = psum.tile([128, R], f32, tag="gemm", bufs=2)
                for q in range(16):
                    nc.tensor.matmul(ph[:], w1all[:, h, q, :], pooledQ[:, q, :],
                                     start=(q == 0), stop=(q == 15))
                if h % 2 == 0:
                    nc.scalar.activation(parts[:, h, :], ph[:], AF.Copy)
                else:
                    nc.vector.tensor_copy(parts[:, h, :], ph[:])
                nc.sync.dma_start(cc_in[h], parts[:, h, :])

            # ---- exchange partials: head h -> core h ----
            cc_out = dpool.tile([8, 128, R], bf16)
            nc.gpsimd.collective_compute(
                "AllToAll", mybir.AluOpType.bypass,
                ins=[cc_in[:]], outs=[cc_out[:]],
                replica_groups=[list(range(N_CORES))],
            )
            hsum = work.tile([128, 8, R], bf16, tag="hsum", bufs=1)
            nc.sync.dma_start(hsum[:], cc_out.rearrange("s p r -> p s r"))

            ADD = mybir.AluOpType.add
            with nc.allow_low_precision(reason="bf16 partial tree"):
                red4 = work.tile([128, 4, R], bf16, tag="red4", bufs=1)
                nc.vector.tensor_tensor(out=red4[:], in0=hsum[:, 0:4, :],
                                        in1=hsum[:, 4:8, :], op=ADD)
                red2 = work.tile([128, 2, R], bf16, tag="red2", bufs=1)
                nc.vector.tensor_tensor(out=red2[:], in0=red4[:, 0:2, :],
                                        in1=red4[:, 2:4, :], op=ADD)
                hfull = work.tile([128, R], f32, tag="hfull", bufs=1)
                nc.vector.tensor_tensor(out=hfull[:], in0=red2[:, 0, :],
                                        in1=red2[:, 1, :], op=ADD)
            hrelu = work.tile([128, R], bf16, tag="hrelu", bufs=1)
            nc.scalar.activation(hrelu[:], hfull[:], AF.Relu, bias=b1c_sb[:])

            po = psum.tile([OUTW, R], f32, tag="head", bufs=1)
            nc.tensor.matmul(po[:], w2_sb[:], hrelu[:], start=True, stop=True)
            om = work.tile([OUTW, R], f32, tag="om", bufs=1)
            nc.vector.scalar_tensor_tensor(
                out=om[:], in0=po[:], scalar=b2_sb[:], in1=keep_sb[:],
                op0=ADD, op1=mybir.AluOpType.mult)
            nc.sync.dma_start(out[:], om[:])
    nc.compile()
    return nc


def _host_prep(preds, image, W_conv, b_conv, W1, b1, W2p, b2p, W2a, b2a, W2d, b2d):
    p = preds[0].astype(np.float32)
    score = p[:, 4] * p[:, 5]
    masked = np.where(score > CONF, score, -np.inf)
    idx = np.argsort(-masked, kind="stable")[:K]
    top_s = masked[idx]
    xy, wh = p[:, 0:2], p[:, 2:4]
    boxes = np.concatenate([xy - wh / 2, xy + wh / 2], axis=-1)
    b = boxes[idx]
    valid = top_s > CONF
    x1, y1, x2, y2 = b[:, 0], b[:, 1], b[:, 2], b[:, 3]
    area = (x2 - x1) * (y2 - y1)
    iw = np.clip(np.minimum(x2[:, None], x2[None, :]) - np.maximum(x1[:, None], x1[None, :]), 0, None)
    ih = np.clip(np.minimum(y2[:, None], y2[None, :]) - np.maximum(y1[:, None], y1[None, :]), 0, None)
    iou = iw * ih / (area[:, None] + area[None, :] - iw * ih + 1e-7)
    keep = valid.copy()
    for i in range(K):
        sup = np.any((iou[i, :i] > IOU) & keep[:i])
        keep[i] = keep[i] & ~sup

    xi = np.clip(np.round(b[:, 0]).astype(np.int32), 0, IMG - INP)
    yi = np.clip(np.round(b[:, 1]).astype(np.int32), 0, IMG - INP)
    img0 = image[0]
    P = np.zeros((R, 3, 66, 66), np.float32)
    for r in range(K):
        P[r, :, 1:65, 1:65] = img0[:, yi[r]:yi[r] + 64, xi[r]:xi[r] + 64]

    from numpy.lib.stride_tricks import sliding_window_view
    # Wv[r, c, yy(64+2), x(64), kx] = P[r, c, yy, x+kx]
    Wv = sliding_window_view(P, 3, axis=3)
    # x column order (xpar, xh): col j -> x = 2*(j%32) + j//32
    xorder = np.array([2 * (j % 32) + j // 32 for j in range(64)])
    t_all = np.empty((8, 91, R, 64), BF16)
    for c in range(8):
        sl = Wv[:, :, 8 * c:8 * c + 10, :, :]          # [R,3,10,64,3]
        tc_ = sl.transpose(2, 1, 4, 0, 3).reshape(90, R, 64)
        t_all[c, :90] = tc_[:, :, xorder].astype(BF16)
        t_all[c, 90] = np.ones((R, 64), BF16)          # bias row

    # stationary conv weights [91=(y,c,kx)+bias, 128=(b,a,oc)], zero-padded
    W91 = np.zeros((91, 128), np.float32)
    for yy in range(10):
        for ci in range(3):
            for kx in range(3):
                row = yy * 9 + ci * 3 + kx
                for bb in range(2):
                    for a in range(4):
                        ky = yy - 2 * a - bb
                        if 0 <= ky < 3:
                            for oc in range(16):
                                W91[row, bb * 64 + a * 16 + oc] = W_conv[oc, ci, ky, kx]
    W91[90, :] = np.tile(b_conv.astype(np.float32), 8)
    W91 = W91.astype(BF16)

    # w1s[core][h, p=(xbit,a,oc), q, d] = W1[h, oc*1024+(4c+a)*32+xbit*16+q, d]
    W1r = W1.reshape(8, 16, 32, 32, 128)  # [h, oc, py, px, d]
    w1s_all = np.empty((8, 8, 128, 16, 128), BF16)
    for c in range(8):
        blk = W1r[:, :, 4 * c:4 * c + 4, :, :]          # [h, oc, a, px, d]
        blk = blk.reshape(8, 16, 4, 2, 16, 128)          # [h, oc, a, xbit, q, d]
        t = blk.transpose(0, 3, 2, 1, 4, 5)              # [h, xbit, a, oc, q, d]
        w1s_all[c] = t.reshape(8, 128, 16, 128).astype(BF16)

    w2_all = np.zeros((8, 128, OUTW), np.float32)
    b2_all = np.zeros((8, OUTW, 1), np.float32)
    w2_all[0, :, :PROV] = W2p; b2_all[0, :PROV, 0] = b2p
    w2_all[1, :, :ALPHA] = W2a; b2_all[1, :ALPHA, 0] = b2a
    for j in range(6):
        w2_all[2 + j, :, :AD] = W2d[j]; b2_all[2 + j, :AD, 0] = b2d[j]

    keepf = np.zeros((R,), np.float32)
    keepf[:K] = keep.astype(np.float32)
    keepm = np.broadcast_to(keepf, (OUTW, R)).copy()

    in_maps = []
    for c in range(8):
        in_maps.append({
            "tmov": t_all[c],
            "w91": W91,
            "w1s": w1s_all[c],
            "b1c": b1[c].reshape(128, 1).astype(np.float32),
            "w2": w2_all[c].astype(BF16),
            "b2": b2_all[c],
            "keepm": keepm,
        })
    return in_maps


def kernel(**inputs):
    from concourse import bass_utils
    if "nc" not in _CACHE:
        _CACHE["nc"] = _build_bass()
    nc = _CACHE["nc"]
    in_maps = _host_prep(**{k: np.asarray(v) for k, v in inputs.items()})
    res = bass_utils.run_bass_kernel_spmd(nc, in_maps, core_ids=list(range(N_CORES)))
    _CACHE["last_res"] = res
    outs = [res.results[c]["out"].T for c in range(N_CORES)]  # [304, 40] each
    logits = np.concatenate(
        [outs[0][:K, :PROV], outs[1][:K, :ALPHA]]
        + [outs[2 + j][:K, :AD] for j in range(6)], axis=1)
    return logits.astype(np.float32)


# revision 19
# speedup vs baseline: 1.2582x; 1.2582x over previous
"""Trainium2 Bass kernel for nn_CombinedModel (NMS detection + ROI classifier).

Strategy (v2, bf16):
  NMS/top-k/ROI selection host-side (tiny). Conv+pool+8-head GEMM on 8 cores.

  Sharding: pooled-row sharding. Core c computes conv rows y in [8c,8c+8)
  (pooled rows py in [4c,4c+4)) of all 304 (padded) ROIs == the k-slice
  {(oc,py,px): py in [4c,4c+4)} (2048 wide) of the 16384-deep W1 contraction.

  Conv as ONE 91-deep matmul per 512 cols: contraction rows = (y10, c3, kx3)
  plus an all-ones row that carries b_conv, with a zero-padded stationary
  [91, 128] whose output columns are (b2, a4, oc16) (y'=2a+b). The moving
  tensor t[(y,c,kx), r, x] holds kx-shifted image rows. 19 PSUM tiles
  [128, 16rois*32xh*2xpar].

  Pool: xpair-max drain from PSUM (DVE tensor_reduce or Act relu + max),
  then fused b-max+relu (scalar_tensor_tensor max0/max) into the two
  pooled2 partition halves (px bit 4).
  pooled2 [128=(px>=16, a, oc), 304 r, 16 q=px%16] bf16.

  GEMM: per (h,q): matmul(w1[128,128] stationary, pooled2[:,:,q] moving),
  16 q accumulated in PSUM; partials [128,8h,304] cast to bf16, exchanged
  with an AllToAll (head h partials -> core h), summed locally, then
  relu+b1, per-core head matmul [128,40], +b2, *keep.
"""
import os
import numpy as np
import ml_dtypes

BF16 = ml_dtypes.bfloat16

N_CORES = 8
R = 304            # 300 rois padded to 19*16
NRB = 19           # conv row blocks of 16 rois
IMG = 640
INP = 64
CONF = 0.25
IOU = 0.45
K = 300
PROV, ALPHA, AD = 38, 25, 35
OUTW = 40          # padded per-core head width

_CACHE = {}


def _build_bass():
    import concourse.bacc as bacc
    import concourse.mybir as mybir
    import concourse.tile as tile

    nc = bacc.Bacc("TRN2", target_bir_lowering=False, debug=False,
                   num_devices=N_CORES)
    f32 = mybir.dt.float32
    bf16 = mybir.dt.bfloat16
    AF = mybir.ActivationFunctionType
    MAX = mybir.AluOpType.max

    tmov = nc.dram_tensor("tmov", [91, R, 64], bf16, kind="ExternalInput").ap()
    w91 = nc.dram_tensor("w91", [91, 128], bf16, kind="ExternalInput").ap()
    w1s = nc.dram_tensor("w1s", [8, 128, 16, 128], bf16, kind="ExternalInput").ap()
    b1c = nc.dram_tensor("b1c", [128, 1], f32, kind="ExternalInput").ap()
    w2 = nc.dram_tensor("w2", [128, OUTW], bf16, kind="ExternalInput").ap()
    b2 = nc.dram_tensor("b2", [OUTW, 1], f32, kind="ExternalInput").ap()
    keepm = nc.dram_tensor("keepm", [OUTW, R], f32, kind="ExternalInput").ap()
    out = nc.dram_tensor("out", [OUTW, R], f32, kind="ExternalOutput").ap()

    with tile.TileContext(nc) as tc:
        with (
            tc.tile_pool(name="const", bufs=1) as cpool,
            tc.tile_pool(name="psum", bufs=1, space="PSUM") as psum,
            tc.tile_pool(name="work", bufs=2) as work,
            tc.tile_pool(name="dram", bufs=1, space="DRAM") as dpool,
        ):
            # conv stationary + first moving chunks go first: they gate rblk 0
            qeng = [nc.sync, nc.scalar]
            w91_sb = cpool.tile([91, 128], bf16)
            nc.sync.dma_start(w91_sb[:], w91[:])
            t_sb = cpool.tile([91, R * 64], bf16)
            rchunks = [(0, 16), (16, 16), (32, 32), (64, 48), (112, 48),
                       (160, 48), (208, 48), (256, 48)]
            for i, (r0, nr) in enumerate(rchunks[:2]):
                qeng[i % 2].dma_start(t_sb[:, r0 * 64:(r0 + nr) * 64],
                                      tmov[:, r0:r0 + nr, :])
            b1c_sb = cpool.tile([128, 1], f32)
            nc.scalar.dma_start(b1c_sb[:], b1c[:])
            w2_sb = cpool.tile([128, OUTW], bf16)
            nc.scalar.dma_start(w2_sb[:], w2[:])
            b2_sb = cpool.tile([OUTW, 1], f32)
            nc.scalar.dma_start(b2_sb[:], b2[:])
            keep_sb = cpool.tile([OUTW, R], f32)
            nc.scalar.dma_start(keep_sb[:], keepm[:])
            for i, (r0, nr) in enumerate(rchunks[2:]):
                qeng[i % 2].dma_start(t_sb[:, r0 * 64:(r0 + nr) * 64],
                                      tmov[:, r0:r0 + nr, :])
            # all 8 heads' W1 slices resident
            w1all = cpool.tile([128, 8, 16, 128], bf16)
            for h in range(8):
                qeng[(h + 1) % 2].dma_start(w1all[:, h, :, :], w1s[h])

            pooled2 = cpool.tile([128, R, 16], bf16)
            pooledQ = cpool.tile([128, 16, R], bf16)
            parts = cpool.tile([128, 8, R], bf16)

            # ---- conv + maxpool + relu ----
            # ps free layout per rblk: (r16, xpar2, xh32)
            for j in range(NRB):
                ps = psum.tile([128, 16, 2, 32], f32, tag="cv", bufs=2)
                psf = ps.rearrange("p a b c -> p (a b c)")
                nc.tensor.matmul(psf[:, 0:512], w91_sb[:],
                                 t_sb[:, j * 1024:j * 1024 + 512],
                                 start=True, stop=True)
                nc.tensor.matmul(psf[:, 512:1024], w91_sb[:],
                                 t_sb[:, j * 1024 + 512:(j + 1) * 1024],
                                 start=True, stop=True)
                # drain PSUM -> bf16 SBUF with relu (contiguous Act write)
                craw = work.tile([128, 16, 2, 32], bf16, tag="craw")
                nc.scalar.activation(
                    craw.rearrange("p a b c -> p (a b c)"), psf[:], AF.Relu)
                # xpair max per partition half (packed bf16, DVE 2x)
                xmA = work.tile([64, 16, 32], bf16, tag="xmA")
                xmB = work.tile([64, 16, 32], bf16, tag="xmB")
                nc.vector.tensor_tensor(out=xmA[:], in0=craw[0:64, :, 0, :],
                                        in1=craw[0:64, :, 1, :], op=MAX)
                nc.vector.tensor_tensor(out=xmB[:], in0=craw[64:128, :, 0, :],
                                        in1=craw[64:128, :, 1, :], op=MAX)
                # b-max into the two pooled2 partition halves (r-major, packed)
                nc.vector.tensor_tensor(
                    out=pooled2[0:64, 16 * j:16 * j + 16, :],
                    in0=xmA[:, :, 0:16], in1=xmB[:, :, 0:16], op=MAX)
                nc.vector.tensor_tensor(
                    out=pooled2[64:128, 16 * j:16 * j + 16, :],
                    in0=xmA[:, :, 16:32], in1=xmB[:, :, 16:32], op=MAX)
                # transpose this rblk to q-major for the GEMM (idle Pool engine)
                nc.gpsimd.tensor_copy(
                    pooledQ[:, :, 16 * j:16 * j + 16],
                    pooled2[:, 16 * j:16 * j + 16, :].rearrange("p r q -> p q r"))

            # ---- 8-head GEMM over this core's 2048-deep W1 slice ----
            cc_in = dpool.tile([8, 128, R], bf16)
            for h in range(8):
                ph = psum.tile([128, R], f32, tag="gemm", bufs=2)
                for q in range(16):
                    nc.tensor.matmul(ph[:], w1all[:, h, q, :], pooledQ[:, q, :],
                                     start=(q == 0), stop=(q == 15))
                if h % 2 == 0:
                    nc.scalar.activation(parts[:, h, :], ph[:], AF.Copy)
                else:
                    nc.vector.tensor_copy(parts[:, h, :], ph[:])
                nc.sync.dma_start(cc_in[h], parts[:, h, :])

            # ---- exchange+reduce partials: head h -> core h ----
            ADD = mybir.AluOpType.add
            cc_out = dpool.tile([128, R], bf16)
            with nc.allow_low_precision(reason="bf16 CCE reduce"):
                nc.gpsimd.collective_compute(
                    "ReduceScatter", ADD,
                    ins=[cc_in[:]], outs=[cc_out[:]],
                    replica_groups=[list(range(N_CORES))],
                )
            hsb = work.tile([128, R], bf16, tag="hsb", bufs=1)
            nc.sync.dma_start(hsb[:], cc_out[:])
            hrelu = work.tile([128, R], bf16, tag="hrelu", bufs=1)
            nc.scalar.activation(hrelu[:], hsb[:], AF.Relu, bias=b1c_sb[:])

            po = psum.tile([OUTW, R], f32, tag="head", bufs=1)
            nc.tensor.matmul(po[:], w2_sb[:], hrelu[:], start=True, stop=True)
            om = work.tile([OUTW, R], f32, tag="om", bufs=1)
            nc.vector.scalar_tensor_tensor(
                out=om[:], in0=po[:], scalar=b2_sb[:], in1=keep_sb[:],
                op0=ADD, op1=mybir.AluOpType.mult)
            nc.sync.dma_start(out[:], om[:])
    nc.compile()
    return nc


def _host_prep(preds, image, W_conv, b_conv, W1, b1, W2p, b2p, W2a, b2a, W2d, b2d):
    p = preds[0].astype(np.float32)
    score = p[:, 4] * p[:, 5]
    masked = np.where(score > CONF, score, -np.inf)
    idx = np.argsort(-masked, kind="stable")[:K]
    top_s = masked[idx]
    xy, wh = p[:, 0:2], p[:, 2:4]
    boxes = np.concatenate([xy - wh / 2, xy + wh / 2], axis=-1)
    b = boxes[idx]
    valid = top_s > CONF
    x1, y1, x2, y2 = b[:, 0], b[:, 1], b[:, 2], b[:, 3]
    area = (x2 - x1) * (y2 - y1)
    iw = np.clip(np.minimum(x2[:, None], x2[None, :]) - np.maximum(x1[:, None], x1[None, :]), 0, None)
    ih = np.clip(np.minimum(y2[:, None], y2[None, :]) - np.maximum(y1[:, None], y1[None, :]), 0, None)
    iou = iw * ih / (area[:, None] + area[None, :] - iw * ih + 1e-7)
    keep = valid.copy()
    for i in range(K):
        sup = np.any((iou[i, :i] > IOU) & keep[:i])
        keep[i] = keep[i] & ~sup

    xi = np.clip(np.round(b[:, 0]).astype(np.int32), 0, IMG - INP)
    yi = np.clip(np.round(b[:, 1]).astype(np.int32), 0, IMG - INP)
    img0 = image[0]
    P = np.zeros((R, 3, 66, 66), np.float32)
    for r in range(K):
        P[r, :, 1:65, 1:65] = img0[:, yi[r]:yi[r] + 64, xi[r]:xi[r] + 64]

    from numpy.lib.stride_tricks import sliding_window_view
    # Wv[r, c, yy(64+2), x(64), kx] = P[r, c, yy, x+kx]
    Wv = sliding_window_view(P, 3, axis=3)
    # x column order (xpar, xh): col j -> x = 2*(j%32) + j//32
    xorder = np.array([2 * (j % 32) + j // 32 for j in range(64)])
    t_all = np.empty((8, 91, R, 64), BF16)
    for c in range(8):
        sl = Wv[:, :, 8 * c:8 * c + 10, :, :]          # [R,3,10,64,3]
        tc_ = sl.transpose(2, 1, 4, 0, 3).reshape(90, R, 64)
        t_all[c, :90] = tc_[:, :, xorder].astype(BF16)
        t_all[c, 90] = np.ones((R, 64), BF16)          # bias row

    # stationary conv weights [91=(y,c,kx)+bias, 128=(b,a,oc)], zero-padded
    W91 = np.zeros((91, 128), np.float32)
    for yy in range(10):
        for ci in range(3):
            for kx in range(3):
                row = yy * 9 + ci * 3 + kx
                for bb in range(2):
                    for a in range(4):
                        ky = yy - 2 * a - bb
                        if 0 <= ky < 3:
                            for oc in range(16):
                                W91[row, bb * 64 + a * 16 + oc] = W_conv[oc, ci, ky, kx]
    W91[90, :] = np.tile(b_conv.astype(np.float32), 8)
    W91 = W91.astype(BF16)

    # w1s[core][h, p=(xbit,a,oc), q, d] = W1[h, oc*1024+(4c+a)*32+xbit*16+q, d]
    W1r = W1.reshape(8, 16, 32, 32, 128)  # [h, oc, py, px, d]
    w1s_all = np.empty((8, 8, 128, 16, 128), BF16)
    for c in range(8):
        blk = W1r[:, :, 4 * c:4 * c + 4, :, :]          # [h, oc, a, px, d]
        blk = blk.reshape(8, 16, 4, 2, 16, 128)          # [h, oc, a, xbit, q, d]
        t = blk.transpose(0, 3, 2, 1, 4, 5)              # [h, xbit, a, oc, q, d]
        w1s_all[c] = t.reshape(8, 128, 16, 128).astype(BF16)

    w2_all = np.zeros((8, 128, OUTW), np.float32)
    b2_all = np.zeros((8, OUTW, 1), np.float32)
    w2_all[0, :, :PROV] = W2p; b2_all[0, :PROV, 0] = b2p
    w2_all[1, :, :ALPHA] = W2a; b2_all[1, :ALPHA, 0] = b2a
    for j in range(6):
        w2_all[2 + j, :, :AD] = W2d[j]; b2_all[2 + j, :AD, 0] = b2d[j]

    keepf = np.zeros((R,), np.float32)
    keepf[:K] = keep.astype(np.float32)
    keepm = np.broadcast_to(keepf, (OUTW, R)).copy()

    in_maps = []
    for c in range(8):
        in_maps.append({
            "tmov": t_all[c],
            "w91": W91,
            "w1s": w1s_all[c],
            "b1c": b1[c].reshape(128, 1).astype(np.float32),
            "w2": w2_all[c].astype(BF16),
            "b2": b2_all[c],
            "keepm": keepm,
        })
    return in_maps


def kernel(**inputs):
    from concourse import bass_utils
    if "nc" not in _CACHE:
        _CACHE["nc"] = _build_bass()
    nc = _CACHE["nc"]
    in_maps = _host_prep(**{k: np.asarray(v) for k, v in inputs.items()})
    res = bass_utils.run_bass_kernel_spmd(nc, in_maps, core_ids=list(range(N_CORES)))
    _CACHE["last_res"] = res
    outs = [res.results[c]["out"].T for c in range(N_CORES)]  # [304, 40] each
    logits = np.concatenate(
        [outs[0][:K, :PROV], outs[1][:K, :ALPHA]]
        + [outs[2 + j][:K, :AD] for j in range(6)], axis=1)
    return logits.astype(np.float32)


# revision 20
# speedup vs baseline: 1.2774x; 1.0152x over previous
"""Trainium2 Bass kernel for nn_CombinedModel (NMS detection + ROI classifier).

Strategy (v2, bf16):
  NMS/top-k/ROI selection host-side (tiny). Conv+pool+8-head GEMM on 8 cores.

  Sharding: pooled-row sharding. Core c computes conv rows y in [8c,8c+8)
  (pooled rows py in [4c,4c+4)) of all 304 (padded) ROIs == the k-slice
  {(oc,py,px): py in [4c,4c+4)} (2048 wide) of the 16384-deep W1 contraction.

  Conv as ONE 91-deep matmul per 512 cols: contraction rows = (y10, c3, kx3)
  plus an all-ones row that carries b_conv, with a zero-padded stationary
  [91, 128] whose output columns are (b2, a4, oc16) (y'=2a+b). The moving
  tensor t[(y,c,kx), r, x] holds kx-shifted image rows. 19 PSUM tiles
  [128, 16rois*32xh*2xpar].

  Pool: xpair-max drain from PSUM (DVE tensor_reduce or Act relu + max),
  then fused b-max+relu (scalar_tensor_tensor max0/max) into the two
  pooled2 partition halves (px bit 4).
  pooled2 [128=(px>=16, a, oc), 304 r, 16 q=px%16] bf16.

  GEMM: per (h,q): matmul(w1[128,128] stationary, pooled2[:,:,q] moving),
  16 q accumulated in PSUM; partials [128,8h,304] cast to bf16, exchanged
  with an AllToAll (head h partials -> core h), summed locally, then
  relu+b1, per-core head matmul [128,40], +b2, *keep.
"""
import os
import numpy as np
import ml_dtypes

BF16 = ml_dtypes.bfloat16

N_CORES = 8
R = 304            # 300 rois padded to 19*16
NRB = 19           # conv row blocks of 16 rois
IMG = 640
INP = 64
CONF = 0.25
IOU = 0.45
K = 300
PROV, ALPHA, AD = 38, 25, 35
OUTW = 40          # padded per-core head width

_CACHE = {}


def _build_bass():
    import concourse.bacc as bacc
    import concourse.mybir as mybir
    import concourse.tile as tile

    nc = bacc.Bacc("TRN2", target_bir_lowering=False, debug=False,
                   num_devices=N_CORES)
    f32 = mybir.dt.float32
    bf16 = mybir.dt.bfloat16
    AF = mybir.ActivationFunctionType
    MAX = mybir.AluOpType.max

    tmov = nc.dram_tensor("tmov", [91, R, 64], bf16, kind="ExternalInput").ap()
    w91 = nc.dram_tensor("w91", [91, 128], bf16, kind="ExternalInput").ap()
    w1s = nc.dram_tensor("w1s", [8, 128, 16, 128], bf16, kind="ExternalInput").ap()
    b1c = nc.dram_tensor("b1c", [128, 1], f32, kind="ExternalInput").ap()
    w2 = nc.dram_tensor("w2", [128, OUTW], bf16, kind="ExternalInput").ap()
    b2 = nc.dram_tensor("b2", [OUTW, 1], f32, kind="ExternalInput").ap()
    keepm = nc.dram_tensor("keepm", [OUTW, R], f32, kind="ExternalInput").ap()
    out = nc.dram_tensor("out", [OUTW, R], f32, kind="ExternalOutput").ap()

    with tile.TileContext(nc) as tc:
        with (
            tc.tile_pool(name="const", bufs=1) as cpool,
            tc.tile_pool(name="psum", bufs=1, space="PSUM") as psum,
            tc.tile_pool(name="work", bufs=2) as work,
            tc.tile_pool(name="dram", bufs=1, space="DRAM") as dpool,
        ):
            # conv stationary + first moving chunks go first: they gate rblk 0
            qeng = [nc.sync, nc.scalar]
            w91_sb = cpool.tile([91, 128], bf16)
            nc.sync.dma_start(w91_sb[:], w91[:])
            t_sb = cpool.tile([91, R * 64], bf16)
            rchunks = [(0, 32), (32, 32), (64, 48), (112, 48),
                       (160, 48), (208, 48), (256, 48)]
            for i, (r0, nr) in enumerate(rchunks[:2]):
                qeng[i % 2].dma_start(t_sb[:, r0 * 64:(r0 + nr) * 64],
                                      tmov[:, r0:r0 + nr, :])
            b1c_sb = cpool.tile([128, 1], f32)
            nc.sync.dma_start(b1c_sb[:], b1c[:])
            w2_sb = cpool.tile([128, OUTW], bf16)
            nc.sync.dma_start(w2_sb[:], w2[:])
            b2_sb = cpool.tile([OUTW, 1], f32)
            nc.sync.dma_start(b2_sb[:], b2[:])
            keep_sb = cpool.tile([OUTW, R], f32)
            nc.sync.dma_start(keep_sb[:], keepm[:])
            for i, (r0, nr) in enumerate(rchunks[2:]):
                qeng[i % 2].dma_start(t_sb[:, r0 * 64:(r0 + nr) * 64],
                                      tmov[:, r0:r0 + nr, :])
            # all 8 heads' W1 slices resident
            w1all = cpool.tile([128, 8, 16, 128], bf16)
            for h in range(8):
                qeng[(h + 1) % 2].dma_start(w1all[:, h, :, :], w1s[h])

            pooled2 = cpool.tile([128, R, 16], bf16)
            pooledQ = cpool.tile([128, 16, R], bf16)
            parts = cpool.tile([128, 8, R], bf16)

            # ---- conv + maxpool + relu ----
            # ps free layout per rblk: (r16, xpar2, xh32)
            for j in range(NRB):
                ps = psum.tile([128, 16, 2, 32], f32, tag="cv", bufs=2)
                psf = ps.rearrange("p a b c -> p (a b c)")
                nc.tensor.matmul(psf[:, 0:512], w91_sb[:],
                                 t_sb[:, j * 1024:j * 1024 + 512],
                                 start=True, stop=True)
                nc.tensor.matmul(psf[:, 512:1024], w91_sb[:],
                                 t_sb[:, j * 1024 + 512:(j + 1) * 1024],
                                 start=True, stop=True)
                # drain PSUM -> bf16 SBUF with relu (contiguous Act write)
                craw = work.tile([128, 16, 2, 32], bf16, tag="craw")
                nc.scalar.activation(
                    craw.rearrange("p a b c -> p (a b c)"), psf[:], AF.Relu)
                # xpair max per partition half (packed bf16, DVE 2x)
                xmA = work.tile([64, 16, 32], bf16, tag="xmA")
                xmB = work.tile([64, 16, 32], bf16, tag="xmB")
                nc.vector.tensor_tensor(out=xmA[:], in0=craw[0:64, :, 0, :],
                                        in1=craw[0:64, :, 1, :], op=MAX)
                nc.vector.tensor_tensor(out=xmB[:], in0=craw[64:128, :, 0, :],
                                        in1=craw[64:128, :, 1, :], op=MAX)
                # b-max into the two pooled2 partition halves (r-major, packed)
                nc.vector.tensor_tensor(
                    out=pooled2[0:64, 16 * j:16 * j + 16, :],
                    in0=xmA[:, :, 0:16], in1=xmB[:, :, 0:16], op=MAX)
                nc.vector.tensor_tensor(
                    out=pooled2[64:128, 16 * j:16 * j + 16, :],
                    in0=xmA[:, :, 16:32], in1=xmB[:, :, 16:32], op=MAX)
                # transpose this rblk to q-major for the GEMM (idle Pool engine)
                nc.gpsimd.tensor_copy(
                    pooledQ[:, :, 16 * j:16 * j + 16],
                    pooled2[:, 16 * j:16 * j + 16, :].rearrange("p r q -> p q r"))

            # ---- 8-head GEMM over this core's 2048-deep W1 slice ----
            cc_in = dpool.tile([8, 128, R], bf16)
            for h in range(8):
                ph = psum.tile([128, R], f32, tag="gemm", bufs=2)
                for q in range(16):
                    nc.tensor.matmul(ph[:], w1all[:, h, q, :], pooledQ[:, q, :],
                                     start=(q == 0), stop=(q == 15))
                if h % 2 == 0:
                    nc.scalar.activation(parts[:, h, :], ph[:], AF.Copy)
                else:
                    nc.vector.tensor_copy(parts[:, h, :], ph[:])
                nc.sync.dma_start(cc_in[h], parts[:, h, :])

            # ---- exchange+reduce partials: head h -> core h ----
            ADD = mybir.AluOpType.add
            cc_out = dpool.tile([128, R], bf16)
            with nc.allow_low_precision(reason="bf16 CCE reduce"):
                nc.gpsimd.collective_compute(
                    "ReduceScatter", ADD,
                    ins=[cc_in[:]], outs=[cc_out[:]],
                    replica_groups=[list(range(N_CORES))],
                )
            hsb = work.tile([128, R], bf16, tag="hsb", bufs=1)
            nc.sync.dma_start(hsb[:], cc_out[:])
            hrelu = work.tile([128, R], bf16, tag="hrelu", bufs=1)
            nc.scalar.activation(hrelu[:], hsb[:], AF.Relu, bias=b1c_sb[:])

            po = psum.tile([OUTW, R], f32, tag="head", bufs=1)
            nc.tensor.matmul(po[:], w2_sb[:], hrelu[:], start=True, stop=True)
            om = work.tile([OUTW, R], f32, tag="om", bufs=1)
            nc.vector.scalar_tensor_tensor(
                out=om[:], in0=po[:], scalar=b2_sb[:], in1=keep_sb[:],
                op0=ADD, op1=mybir.AluOpType.mult)
            nc.sync.dma_start(out[:], om[:])
    nc.compile()
    return nc


def _host_prep(preds, image, W_conv, b_conv, W1, b1, W2p, b2p, W2a, b2a, W2d, b2d):
    p = preds[0].astype(np.float32)
    score = p[:, 4] * p[:, 5]
    masked = np.where(score > CONF, score, -np.inf)
    idx = np.argsort(-masked, kind="stable")[:K]
    top_s = masked[idx]
    xy, wh = p[:, 0:2], p[:, 2:4]
    boxes = np.concatenate([xy - wh / 2, xy + wh / 2], axis=-1)
    b = boxes[idx]
    valid = top_s > CONF
    x1, y1, x2, y2 = b[:, 0], b[:, 1], b[:, 2], b[:, 3]
    area = (x2 - x1) * (y2 - y1)
    iw = np.clip(np.minimum(x2[:, None], x2[None, :]) - np.maximum(x1[:, None], x1[None, :]), 0, None)
    ih = np.clip(np.minimum(y2[:, None], y2[None, :]) - np.maximum(y1[:, None], y1[None, :]), 0, None)
    iou = iw * ih / (area[:, None] + area[None, :] - iw * ih + 1e-7)
    keep = valid.copy()
    for i in range(K):
        sup = np.any((iou[i, :i] > IOU) & keep[:i])
        keep[i] = keep[i] & ~sup

    xi = np.clip(np.round(b[:, 0]).astype(np.int32), 0, IMG - INP)
    yi = np.clip(np.round(b[:, 1]).astype(np.int32), 0, IMG - INP)
    img0 = image[0]
    P = np.zeros((R, 3, 66, 66), np.float32)
    for r in range(K):
        P[r, :, 1:65, 1:65] = img0[:, yi[r]:yi[r] + 64, xi[r]:xi[r] + 64]

    from numpy.lib.stride_tricks import sliding_window_view
    # Wv[r, c, yy(64+2), x(64), kx] = P[r, c, yy, x+kx]
    Wv = sliding_window_view(P, 3, axis=3)
    # x column order (xpar, xh): col j -> x = 2*(j%32) + j//32
    xorder = np.array([2 * (j % 32) + j // 32 for j in range(64)])
    t_all = np.empty((8, 91, R, 64), BF16)
    for c in range(8):
        sl = Wv[:, :, 8 * c:8 * c + 10, :, :]          # [R,3,10,64,3]
        tc_ = sl.transpose(2, 1, 4, 0, 3).reshape(90, R, 64)
        t_all[c, :90] = tc_[:, :, xorder].astype(BF16)
        t_all[c, 90] = np.ones((R, 64), BF16)          # bias row

    # stationary conv weights [91=(y,c,kx)+bias, 128=(b,a,oc)], zero-padded
    W91 = np.zeros((91, 128), np.float32)
    for yy in range(10):
        for ci in range(3):
            for kx in range(3):
                row = yy * 9 + ci * 3 + kx
                for bb in range(2):
                    for a in range(4):
                        ky = yy - 2 * a - bb
                        if 0 <= ky < 3:
                            for oc in range(16):
                                W91[row, bb * 64 + a * 16 + oc] = W_conv[oc, ci, ky, kx]
    W91[90, :] = np.tile(b_conv.astype(np.float32), 8)
    W91 = W91.astype(BF16)

    # w1s[core][h, p=(xbit,a,oc), q, d] = W1[h, oc*1024+(4c+a)*32+xbit*16+q, d]
    W1r = W1.reshape(8, 16, 32, 32, 128)  # [h, oc, py, px, d]
    w1s_all = np.empty((8, 8, 128, 16, 128), BF16)
    for c in range(8):
        blk = W1r[:, :, 4 * c:4 * c + 4, :, :]          # [h, oc, a, px, d]
        blk = blk.reshape(8, 16, 4, 2, 16, 128)          # [h, oc, a, xbit, q, d]
        t = blk.transpose(0, 3, 2, 1, 4, 5)              # [h, xbit, a, oc, q, d]
        w1s_all[c] = t.reshape(8, 128, 16, 128).astype(BF16)

    w2_all = np.zeros((8, 128, OUTW), np.float32)
    b2_all = np.zeros((8, OUTW, 1), np.float32)
    w2_all[0, :, :PROV] = W2p; b2_all[0, :PROV, 0] = b2p
    w2_all[1, :, :ALPHA] = W2a; b2_all[1, :ALPHA, 0] = b2a
    for j in range(6):
        w2_all[2 + j, :, :AD] = W2d[j]; b2_all[2 + j, :AD, 0] = b2d[j]

    keepf = np.zeros((R,), np.float32)
    keepf[:K] = keep.astype(np.float32)
    keepm = np.broadcast_to(keepf, (OUTW, R)).copy()

    in_maps = []
    for c in range(8):
        in_maps.append({
            "tmov": t_all[c],
            "w91": W91,
            "w1s": w1s_all[c],
            "b1c": b1[c].reshape(128, 1).astype(np.float32),
            "w2": w2_all[c].astype(BF16),
            "b2": b2_all[c],
            "keepm": keepm,
        })
    return in_maps


def kernel(**inputs):
    from concourse import bass_utils
    if "nc" not in _CACHE:
        _CACHE["nc"] = _build_bass()
    nc = _CACHE["nc"]
    in_maps = _host_prep(**{k: np.asarray(v) for k, v in inputs.items()})
    res = bass_utils.run_bass_kernel_spmd(nc, in_maps, core_ids=list(range(N_CORES)))
    _CACHE["last_res"] = res
    outs = [res.results[c]["out"].T for c in range(N_CORES)]  # [304, 40] each
    logits = np.concatenate(
        [outs[0][:K, :PROV], outs[1][:K, :ALPHA]]
        + [outs[2 + j][:K, :AD] for j in range(6)], axis=1)
    return logits.astype(np.float32)
